# revision 1
# baseline (speedup 1.0000x reference)
"""Multi-head attention (B=4, S=2048, D=1024, H=16) on 8 Trainium2 cores.

Sharding: core c computes batch b = c // 2, head group hg = c % 2 (8 heads).
Each core runs the full pipeline for its (b, hg): QKV projections restricted
to its head group's rows of Wq/Wk/Wv, per-head attention, and a partial
output projection against its head group's columns of Wo. The host sums the
two partial outputs per batch (the out-projection is linear in the head dim).

On-device layouts are transposed (feature on partitions):
  qT/kT [512, 2048]: head-dim on partitions; pair tile p holds head 2p on
  partitions 0:64 and head 2p+1 on 64:128, so the two scores matmuls of a
  pair land on disjoint PE row groups and run concurrently.
  Scores are computed as S^T [k, q]; exp(S^T/8) -> E^T feeds the context
  matmul as the *moving* operand with [V | ones] stationary, yielding
  ctx^T [d, q] plus the softmax denominator as row 64 of the same PSUM
  accumulation. Normalization = reciprocal of that row, partition-broadcast
  (GpSimd), one multiply. ctx^T is exactly the lhsT the out-projection
  needs, so no transposes anywhere.
"""

import numpy as np

B, S, D, H = 4, 2048, 1024, 16
HD = D // H          # 64
NHL = 8              # heads per core
DHG = NHL * HD       # 512 head-group width
HDA = HD + 1         # augmented head dim (ones column)
P = 128
N_CORES = 8

_CACHE = {}


def _build_nc(debug=False):
    import concourse.bacc as bacc
    import concourse.mybir as mybir
    from concourse.tile import TileContext

    f16 = mybir.dt.float16
    f32 = mybir.dt.float32
    EXP = mybir.ActivationFunctionType.Exp

    nc = bacc.Bacc("TRN2", target_bir_lowering=False, debug=False,
                   num_devices=N_CORES)

    xqT = nc.dram_tensor("xqT", [D, S], f16, kind="ExternalInput")
    xkT = nc.dram_tensor("xkT", [D, S], f16, kind="ExternalInput")
    xvT = nc.dram_tensor("xvT", [D, S], f16, kind="ExternalInput")
    wqT = nc.dram_tensor("wqT", [D, DHG], f16, kind="ExternalInput")
    wkT = nc.dram_tensor("wkT", [D, DHG], f16, kind="ExternalInput")
    wvT = nc.dram_tensor("wvT", [D, DHG], f16, kind="ExternalInput")
    woT = nc.dram_tensor("woT", [DHG, D], f16, kind="ExternalInput")
    out = nc.dram_tensor("out", [S, D], f32, kind="ExternalOutput")
    if debug:
        dbg_qT = nc.dram_tensor("dbg_qT", [DHG, S], f16, kind="ExternalOutput")
        dbg_kT = nc.dram_tensor("dbg_kT", [DHG, S], f16, kind="ExternalOutput")
        dbg_vaug = nc.dram_tensor("dbg_vaug", [S, NHL * HDA], f16, kind="ExternalOutput")
        dbg_ctxT = nc.dram_tensor("dbg_ctxT", [DHG, S], f16, kind="ExternalOutput")

    DT = D // P          # 8 input-dim tiles
    PT = DHG // P        # 4 head-pair tiles
    QC = S // 512        # 4 query chunks
    KB = S // P          # 16 key chunks

    with TileContext(nc) as tc:
        with (
            tc.tile_pool(name="weights", bufs=1) as wpool,
            tc.tile_pool(name="persist", bufs=1) as persist,
            tc.tile_pool(name="xstream", bufs=16) as xpool,
            tc.tile_pool(name="evict", bufs=6) as epool,
            tc.tile_pool(name="norm", bufs=3) as npool,
            tc.tile_pool(name="proj_psum", bufs=2, space="PSUM") as proj_psum,
            tc.tile_pool(name="sc_psum", bufs=2, space="PSUM") as sc_psum,
            tc.tile_pool(name="ctx_psum", bufs=2, space="PSUM") as ctx_psum,
        ):
            wq = wpool.tile([P, DT, DHG], f16)
            wk = wpool.tile([P, DT, DHG], f16)
            wv = wpool.tile([P, DT, DHG], f16)
            wo = wpool.tile([P, PT, D], f16)
            for dt in range(DT):
                nc.sync.dma_start(wq[:, dt], wqT[dt * P:(dt + 1) * P, :])
                nc.sync.dma_start(wk[:, dt], wkT[dt * P:(dt + 1) * P, :])
                nc.sync.dma_start(wv[:, dt], wvT[dt * P:(dt + 1) * P, :])
            for dt in range(PT):
                nc.sync.dma_start(wo[:, dt], woT[dt * P:(dt + 1) * P, :])

            qT = persist.tile([P, PT, S], f16)
            kT = persist.tile([P, PT, S], f16)
            vaug = persist.tile([P, KB, NHL * HDA], f16)
            ctxT = persist.tile([P, PT, S], f16)

            # K and Q projections.  Each x tile is loaded once (qc-outer)
            # and consumed by all four head-pair output tiles.  K runs
            # first, and Q's pair-0 chunks are emitted first within each
            # qc so attention on pair 0 can start as early as possible.
            def project_kq(w, xdram, dst, only_qc=None):
                for qc in ([only_qc] if only_qc is not None else range(QC)):
                    xs = []
                    for dt in range(DT):
                        xt = xpool.tile([P, 512], f16, tag="x")
                        nc.sync.dma_start(
                            xt[:], xdram[dt * P:(dt + 1) * P,
                                         qc * 512:(qc + 1) * 512])
                        xs.append(xt)
                    for pt in range(PT):
                        ps = proj_psum.tile([P, 512], f32, tag="proj")
                        for dt in range(DT):
                            nc.tensor.matmul(
                                ps[:], w[:, dt, pt * P:(pt + 1) * P], xs[dt][:],
                                start=(dt == 0), stop=(dt == DT - 1))
                        nc.vector.tensor_copy(
                            dst[:, pt, qc * 512:(qc + 1) * 512], ps[:])

            def project_v(kc):
                xs = []
                for dt in range(DT):
                    xt = xpool.tile([P, 512], f16, tag="x")
                    nc.sync.dma_start(
                        xt[:], xvT[dt * P:(dt + 1) * P, kc * 512:(kc + 1) * 512])
                    xs.append(xt)
                for ks in range(4):
                    kb = kc * 4 + ks
                    ps = proj_psum.tile([P, 512], f32, tag="proj")
                    for dt in range(DT):
                        nc.tensor.matmul(
                            ps[:], xs[dt][:, ks * P:(ks + 1) * P], wv[:, dt],
                            start=(dt == 0), stop=(dt == DT - 1))
                    va = vaug[:, kb].rearrange("p (h x) -> p h x", h=NHL)
                    nc.any.memset(va[:, :, HD:HDA], 1.0)
                    nc.vector.tensor_copy(
                        va[:, :, 0:HD],
                        ps[:].rearrange("p (h x) -> p h x", h=NHL))

            # Emission order interleaves the remaining projections with the
            # attention chunks: attention on query chunk qc only needs all
            # of K, Q chunk qc, and V chunks as its key loop reaches them,
            # so ACT starts exponentiating ~50us earlier and the V/Q
            # projection matmuls fill PE slack while ACT is the bottleneck.
            project_kq(wk, xkT, kT)
            for kc in range(QC):
                project_v(kc)
            project_kq(wq, xqT, qT)

            # Attention: qc-outer so each query chunk's output-projection
            # rows can run as soon as all pairs finish that chunk.
            # ctx^T accumulates with [V | ones] stationary: PSUM rows 0:64
            # are ctx^T, row 64 is the softmax denominator per query.
            def outproj_piece(sc_):
                for jc in range(2):
                    ps = proj_psum.tile([P, 512], f32, tag="proj")
                    for dt in range(PT):
                        nc.tensor.matmul(
                            ps[:], ctxT[:, dt, sc_ * P:(sc_ + 1) * P],
                            wo[:, dt, jc * 512:(jc + 1) * 512],
                            start=(dt == 0), stop=(dt == PT - 1))
                    ot = epool.tile([P, 512], f32, tag="o")
                    nc.vector.tensor_copy(ot[:], ps[:])
                    nc.sync.dma_start(
                        out[sc_ * P:(sc_ + 1) * P, jc * 512:(jc + 1) * 512],
                        ot[:])

            for qc in range(QC):
                for pr in range(PT):
                    if qc >= 1:
                        outproj_piece((qc - 1) * 4 + pr)
                    cps = [ctx_psum.tile([P, 512], f32, tag="ctx",
                                         name=f"ctx_{pr}_{qc}_{h}")
                           for h in range(2)]
                    for kb in range(KB):
                        sc = sc_psum.tile([P, 1024], f32, tag="sc")
                        et = epool.tile([P, 1024], f16, tag="e")
                        for h in range(2):
                            nc.tensor.matmul(
                                sc[:, h * 512:(h + 1) * 512],
                                kT[h * 64:(h + 1) * 64, pr, kb * P:(kb + 1) * P],
                                qT[h * 64:(h + 1) * 64, pr, qc * 512:(qc + 1) * 512],
                                start=True, stop=True)
                        nc.scalar.activation(et[:], sc[:], EXP, scale=1.0 / 8.0)
                        for h in range(2):
                            hg = 2 * pr + h
                            nc.tensor.matmul(
                                cps[h][0:HDA, :],
                                vaug[:, kb, hg * HDA:(hg + 1) * HDA],
                                et[:, h * 512:(h + 1) * 512],
                                start=(kb == 0), stop=(kb == KB - 1))
                    for h in range(2):
                        # denominator row lives at PSUM partition 64; the
                        # custom-DVE reciprocal and partition_broadcast only
                        # operate from partition 0, so bounce it via DMA.
                        den = npool.tile([P, 512], f32, tag="den",
                                         name=f"den_{pr}_{qc}_{h}")
                        nc.vector.tensor_copy(den[HD:HDA, :], cps[h][HD:HDA, :])
                        nc.sync.dma_start(den[0:1, :], den[HD:HDA, :])
                        rec = npool.tile([1, 512], f32, tag="rec")
                        nc.vector.reciprocal_approx_fast(rec[0:1, :], den[0:1, :])
                        rb = npool.tile([HD, 512], f32, tag="rb")
                        nc.gpsimd.partition_broadcast(rb[:], rec[0:1, :])
                        if h == 0:
                            nc.vector.tensor_mul(
                                ctxT[0:HD, pr, qc * 512:(qc + 1) * 512],
                                cps[h][0:HD, :], rb[:])
                        else:
                            tmp = npool.tile([HD, 512], f16, tag="tmp")
                            nc.vector.tensor_mul(tmp[:], cps[h][0:HD, :], rb[:])
                            nc.sync.dma_start(
                                ctxT[HD:P, pr, qc * 512:(qc + 1) * 512], tmp[:])

            for sc_ in range(12, 16):
                outproj_piece(sc_)

            if debug:
                for pt in range(PT):
                    nc.sync.dma_start(dbg_qT[pt * P:(pt + 1) * P, :], qT[:, pt])
                    nc.sync.dma_start(dbg_kT[pt * P:(pt + 1) * P, :], kT[:, pt])
                    nc.sync.dma_start(dbg_ctxT[pt * P:(pt + 1) * P, :], ctxT[:, pt])
                for kb in range(KB):
                    nc.sync.dma_start(dbg_vaug[kb * P:(kb + 1) * P, :], vaug[:, kb])

    nc.compile()
    return nc


def _prep_inputs(query, key, value, Wq, Wk, Wv, Wo):
    """Per-core input maps; host does the transposes and fp16 casts."""
    f16 = np.float16
    in_maps = []
    wT = {}
    for hg in range(2):
        lo, hi = hg * DHG, (hg + 1) * DHG
        wT[hg] = {
            "wqT": np.ascontiguousarray(Wq[lo:hi, :].T).astype(f16),
            "wkT": np.ascontiguousarray(Wk[lo:hi, :].T).astype(f16),
            "wvT": np.ascontiguousarray(Wv[lo:hi, :].T).astype(f16),
            "woT": np.ascontiguousarray(Wo[:, lo:hi].T).astype(f16),
        }
    for c in range(N_CORES):
        b, hg = c // 2, c % 2
        in_maps.append({
            "xqT": np.ascontiguousarray(query[b].T).astype(f16),
            "xkT": np.ascontiguousarray(key[b].T).astype(f16),
            "xvT": np.ascontiguousarray(value[b].T).astype(f16),
            **wT[hg],
        })
    return in_maps


def _reference_numpy(query, key, value, mask, Wq, Wk, Wv, Wo):
    """Correctness fallback for inputs the fast path doesn't handle."""
    out = np.empty((B, S, D), np.float32)
    for b in range(B):
        q = (query[b] @ Wq.T).reshape(S, H, HD).transpose(1, 0, 2)
        k = (key[b] @ Wk.T).reshape(S, H, HD).transpose(1, 0, 2)
        v = (value[b] @ Wv.T).reshape(S, H, HD).transpose(1, 0, 2)
        scores = np.einsum("hqd,hkd->hqk", q, k) / np.sqrt(np.float32(HD))
        scores = np.where(mask[b][None, :, :] == 0, -np.inf, scores)
        scores = scores - scores.max(axis=-1, keepdims=True)
        e = np.exp(scores)
        attn = e / e.sum(axis=-1, keepdims=True)
        ctx = np.einsum("hqk,hkd->hqd", attn, v)
        out[b] = ctx.transpose(1, 0, 2).reshape(S, D) @ Wo.T
    return out


def run_device(query, key, value, Wq, Wk, Wv, Wo, trace=False, trace_kwargs=None,
               debug=False):
    from concourse.bass_utils import run_bass_kernel_spmd

    key_ = ("nc", debug)
    if key_ not in _CACHE:
        _CACHE[key_] = _build_nc(debug)
    nc = _CACHE[key_]
    in_maps = _prep_inputs(query, key, value, Wq, Wk, Wv, Wo)
    res = run_bass_kernel_spmd(nc, in_maps, list(range(N_CORES)),
                               trace=trace, **(trace_kwargs or {}))
    out = np.empty((B, S, D), np.float32)
    for b in range(B):
        out[b] = res.results[2 * b]["out"] + res.results[2 * b + 1]["out"]
    return out, res


def kernel(query, key, value, mask, Wq, Wk, Wv, Wo):
    query = np.asarray(query, np.float32)
    key = np.asarray(key, np.float32)
    value = np.asarray(value, np.float32)
    Wq = np.asarray(Wq, np.float32)
    Wk = np.asarray(Wk, np.float32)
    Wv = np.asarray(Wv, np.float32)
    Wo = np.asarray(Wo, np.float32)
    if not np.all(np.asarray(mask) == 1):
        return _reference_numpy(query, key, value, np.asarray(mask),
                                Wq, Wk, Wv, Wo)
    out, _ = run_device(query, key, value, Wq, Wk, Wv, Wo)
    return out



# revision 8
# speedup vs baseline: 1.2552x; 1.2552x over previous
"""Multi-head attention (B=4, S=2048, D=1024, H=16) on 8 Trainium2 cores.

Sharding: core c computes batch b = c // 2, head group hg = c % 2 (8 heads).
Each core runs the full pipeline for its (b, hg); the host sums the two
partial out-projection results per batch (linear in the head dim).

v1 kernel structure (per core):
  * QKV projections in fp8 (e4m3) DoubleRow matmuls: host pre-splits x (*4)
    and W (*128) into hi+lo fp8 pairs; each DoubleRow instruction carries two
    rank-128 contraction slots, and the hi*hi / lo*hi / hi*lo products (lo*lo
    dropped, ~2.6e-4 relative) pack 3 slots per 2 contraction chunks ->
    0.75x the fp16 PE cost. PSUM = 512*(x@W); evictions scale by 1/512.
  * Scores stay fp16 (single rank-64 chunk per output block: DoubleRow slot
    packing cannot help), computed as S^T [k, q] per head pair, exp on ACT
    (scale 1/8) -> E^T tiles [128k, 2h*512q] fp16.
  * Context via stationary-E orientation: out ctx [128 q, 65] = E-chunk^T
    [k,128q]^T-stationary x [V | ones] moving -> PE cost 65 cycles per
    rank-128 key block (vs 512 in the V-stationary orientation). PSUM col 64
    accumulates the softmax denominator; normalization is a [128,1]
    reciprocal + per-partition scalar multiply, written as [q, 2h*64] f16
    and DMA-transposed (xbar) into the ctxT [d, q] layout the out-projection
    needs as its stationary operand.
  * Out-projection fp16 as in v0.
"""

import numpy as np

B, S, D, H = 4, 2048, 1024, 16
HD = D // H          # 64
NHL = 8              # heads per core
DHG = NHL * HD       # 512 head-group width
HDA = HD + 1         # augmented head dim (ones column)
P = 128
N_CORES = 8

SX = 4.0             # host fp8 pre-scale for activations
SW = 128.0           # host fp8 pre-scale for weights
SEV = 1.0 / (SX * SW)  # eviction scale restoring natural units

_CACHE = {}


def _build_nc(debug=False):
    import concourse.bacc as bacc
    import concourse.mybir as mybir
    from concourse.tile import TileContext

    f8 = mybir.dt.float8e4
    f16 = mybir.dt.float16
    f32 = mybir.dt.float32
    EXP = mybir.ActivationFunctionType.Exp
    DR = mybir.MatmulPerfMode.DoubleRow
    MUL = mybir.AluOpType.mult

    nc = bacc.Bacc("TRN2", target_bir_lowering=False, debug=False,
                   num_devices=N_CORES)

    # fp8 hi/lo packed inputs: [...,2,...] pair dim innermost-but-one.
    xq8 = nc.dram_tensor("xq8", [D, 2, S], f8, kind="ExternalInput")
    xk8 = nc.dram_tensor("xk8", [D, 2, S], f8, kind="ExternalInput")
    xv8 = nc.dram_tensor("xv8", [D, 2, S], f8, kind="ExternalInput")
    wq8 = nc.dram_tensor("wq8", [D, 2, DHG], f8, kind="ExternalInput")
    wk8 = nc.dram_tensor("wk8", [D, 2, DHG], f8, kind="ExternalInput")
    wv8 = nc.dram_tensor("wv8", [D, 2, DHG], f8, kind="ExternalInput")
    woT = nc.dram_tensor("woT", [DHG, D], f16, kind="ExternalInput")
    out = nc.dram_tensor("out", [S, D], f32, kind="ExternalOutput")

    DT = D // P          # 8 contraction chunks for projections
    PT = DHG // P        # 4 head-pair tiles
    QC = S // 512        # 4 query chunks
    KB = S // P          # 16 key blocks
    QS = 4               # 128-query sub-chunks per qc

    with TileContext(nc) as tc:
        with (
            tc.tile_pool(name="weights", bufs=1) as wpool,
            tc.tile_pool(name="persist", bufs=1) as persist,
            tc.tile_pool(name="xstream", bufs=6) as xpool,
            tc.tile_pool(name="etile", bufs=34) as etpool,
            tc.tile_pool(name="evict", bufs=5) as epool,
            tc.tile_pool(name="norm", bufs=10) as npool,
            tc.tile_pool(name="ctxsb", bufs=6) as cspool,
            tc.tile_pool(name="proj_psum", bufs=2, space="PSUM") as proj_psum,
            tc.tile_pool(name="sc_psum", bufs=2, space="PSUM") as sc_psum,
            tc.tile_pool(name="ctx_psum", bufs=2, space="PSUM") as ctx_psum,
        ):
            # -- weights ---------------------------------------------------
            wq = [wpool.tile([P, DT, DHG], f8, name=f"wq{i}") for i in range(2)]
            wk = [wpool.tile([P, DT, DHG], f8, name=f"wk{i}") for i in range(2)]
            wv = [wpool.tile([P, DT, DHG], f8, name=f"wv{i}") for i in range(2)]
            wo = wpool.tile([P, PT, D], f16)
            for (wt, wd) in ((wq, wq8), (wk, wk8), (wv, wv8)):
                for i in range(2):
                    nc.sync.dma_start(
                        wt[i][:],
                        wd[:, i, :].rearrange("(dt p) m -> p dt m", p=P))
            nc.sync.dma_start(
                wo[:], woT[:].rearrange("(pt p) m -> p pt m", p=P))

            kT = persist.tile([P, PT, S], f16)
            qT = persist.tile([P, PT, S], f16)
            vaug = persist.tile([P, KB, NHL * HDA], f16)
            ctxT = persist.tile([P, PT, S], f16)
            for kb in range(KB):
                nc.any.memset(
                    vaug[:, kb].rearrange("p (h x) -> p h x", h=NHL)[:, :, HD:HDA],
                    1.0)

            def load_x(xdram, qc):
                """x chunk hi/lo pair: two [128, DT, 512] fp8 tiles."""
                xts = []
                for i in range(2):
                    xt = xpool.tile([P, DT, 512], f8, tag="x", name=f"x{i}")
                    nc.sync.dma_start(
                        xt[:],
                        xdram[:, i, qc * 512:(qc + 1) * 512]
                        .rearrange("(dt p) n -> p dt n", p=P))
                    xts.append(xt)
                return xts

            def mm3(ps, w8, x8, mslice):
                """3-slot hi/lo DoubleRow product into ps over all DT chunks.

                w8/x8 = [hi, lo] tile pairs, each [P, DT, n]; the pair dim of
                each instruction = two adjacent dt chunks of one variant.
                """
                n_inst = DT // 2 * 3
                i = 0
                for c in range(0, DT, 2):
                    for (wi, xi) in ((0, 0), (1, 0), (0, 1)):
                        lhs = w8[wi][:, c:c + 2, mslice]
                        rhs = x8[xi][:, c:c + 2, :]
                        nc.tensor.matmul(
                            ps[:], lhs, rhs,
                            start=(i == 0), stop=(i == n_inst - 1),
                            perf_mode=DR)
                        i += 1

            def project_kq(w8, xdram, dst, prs=range(PT), qcs=range(QC)):
                for qc in qcs:
                    xt = load_x(xdram, qc)
                    for pt in prs:
                        ps = proj_psum.tile([P, 512], f32, tag="proj")
                        mm3(ps, w8, xt, slice(pt * P, (pt + 1) * P))
                        nc.vector.tensor_scalar_mul(
                            dst[:, pt, qc * 512:(qc + 1) * 512], ps[:], SEV)

            def project_v(kc):
                xt = load_x(xv8, kc)
                for ks in range(4):
                    kb = kc * 4 + ks
                    ps = proj_psum.tile([P, 512], f32, tag="proj")
                    # stationary = x chunk pairs, moving = wv pairs
                    n_inst = DT // 2 * 3
                    i = 0
                    for c in range(0, DT, 2):
                        for (xi, wi) in ((0, 0), (1, 0), (0, 1)):
                            lhs = xt[xi][:, c:c + 2, ks * P:(ks + 1) * P]
                            rhs = wv[wi][:, c:c + 2, :]
                            nc.tensor.matmul(
                                ps[:], lhs, rhs,
                                start=(i == 0), stop=(i == n_inst - 1),
                                perf_mode=DR)
                            i += 1
                    nc.vector.tensor_scalar_mul(
                        vaug[:, kb].rearrange("p (h x) -> p h x", h=NHL)[:, :, 0:HD],
                        ps[:].rearrange("p (h x) -> p h x", h=NHL), SEV)

            # -- attention phases -----------------------------------------
            def phase_scores(pr, qc):
                """16 kb: two fp16 scores matmuls + one exp -> et set."""
                ets = []
                for kb in range(KB):
                    sc = sc_psum.tile([P, 1024], f32, tag="sc")
                    et = etpool.tile([P, 1024], f16, tag="e",
                                     name=f"et_{pr}_{qc}_{kb}")
                    for h in range(2):
                        nc.tensor.matmul(
                            sc[:, h * 512:(h + 1) * 512],
                            kT[h * 64:(h + 1) * 64, pr, kb * P:(kb + 1) * P],
                            qT[h * 64:(h + 1) * 64, pr, qc * 512:(qc + 1) * 512],
                            start=True, stop=True)
                    nc.scalar.activation(et[:], sc[:], EXP, scale=1.0 / 8.0)
                    ets.append(et)
                return ets

            def phase_ctx(pr, qc, ets):
                """8 groups (2 heads x 4 qsub): stationary-E ctx + norm."""
                for h in range(2):
                    hg = 2 * pr + h
                    for qs in range(QS):
                        cps = ctx_psum.tile([P, 512], f32, tag="ctx")
                        for kb in range(KB):
                            nc.tensor.matmul(
                                cps[:, 0:HDA],
                                ets[kb][:, h * 512 + qs * P:
                                        h * 512 + (qs + 1) * P],
                                vaug[:, kb, hg * HDA:(hg + 1) * HDA],
                                start=(kb == 0), stop=(kb == KB - 1))
                        rec = npool.tile([P, 1], f32, tag="rec")
                        nc.vector.reciprocal_approx_fast(
                            rec[:], cps[:, HD:HDA])
                        if h == 0:
                            csb = cspool.tile([P, P], f16, tag="csb",
                                              name=f"csb_{pr}_{qc}_{qs}")
                            _csb_stash[(pr, qc, qs)] = csb
                        else:
                            csb = _csb_stash.pop((pr, qc, qs))
                        nc.vector.tensor_scalar_mul(
                            csb[:, h * HD:(h + 1) * HD],
                            cps[:, 0:HD], rec[:])
                        if h == 1:
                            nc.sync.dma_start_transpose(
                                ctxT[:, pr, qc * 512 + qs * P:
                                     qc * 512 + (qs + 1) * P],
                                csb[:])

            _csb_stash = {}

            def outproj_piece(sc_):
                for jc in range(2):
                    ps = proj_psum.tile([P, 512], f32, tag="proj")
                    for dt in range(PT):
                        nc.tensor.matmul(
                            ps[:], ctxT[:, dt, sc_ * P:(sc_ + 1) * P],
                            wo[:, dt, jc * 512:(jc + 1) * 512],
                            start=(dt == 0), stop=(dt == PT - 1))
                    ot = epool.tile([P, 512], f32, tag="o")
                    nc.vector.tensor_copy(ot[:], ps[:])
                    nc.sync.dma_start(
                        out[sc_ * P:(sc_ + 1) * P, jc * 512:(jc + 1) * 512],
                        ot[:])

            # -- emission order -------------------------------------------
            # PE executes strictly in order, so every instruction a phase
            # depends on must be emitted before it.  Startup: K rows for
            # pr0 + Q for qc0, so the first exp lands ~15us in.  V and the
            # remaining K/Q projections are woven into the early score
            # phases (each S phase leaves ~10us of PE slack vs its 16.6us
            # of ACT exp work).  C(pr) follows S(pr+1) - one-phase lag -
            # keeping <=2 E-tile sets alive.  Out-projection pieces drain
            # one per phase-step starting in qc1.
            project_kq(wk, xk8, kT, prs=[0])
            project_kq(wq, xq8, qT, qcs=[0])

            weave = [
                lambda: project_v(0),
                lambda: project_v(1),
                lambda: project_kq(wk, xk8, kT, prs=[1]),
                lambda: project_v(2),
                lambda: project_v(3),
                lambda: project_kq(wk, xk8, kT, prs=[2]),
                lambda: project_kq(wk, xk8, kT, prs=[3]),
                lambda: project_kq(wq, xq8, qT, qcs=[1]),
                None,
                lambda: project_kq(wq, xq8, qT, qcs=[2]),
                None, None, None,
                lambda: project_kq(wq, xq8, qT, qcs=[3]),
            ]
            # per-step weave allotment: 3 units in each of the first two
            # steps (V+K must complete before C(0,0)), then 1 per step.
            allot = [3, 3] + [1] * 14
            wi = 0
            pending_c = []
            op_queue = []
            step = 0
            for qc in range(QC):
                for pr in range(PT):
                    ets = phase_scores(pr, qc)
                    for _ in range(allot[step]):
                        if wi < len(weave):
                            u = weave[wi]
                            wi += 1
                            if u is not None:
                                u()
                    pending_c.append((pr, qc, ets))
                    if step >= 1:
                        # V/K weave for C(0,0) completes during step 1's
                        # weave (emitted just above, before this C).
                        while len(pending_c) > 1:
                            cpr, cqc, cets = pending_c.pop(0)
                            phase_ctx(cpr, cqc, cets)
                    step += 1
                    if op_queue:
                        outproj_piece(op_queue.pop(0))
                if qc < QC - 1:
                    op_queue.extend(range(qc * 4, qc * 4 + 4))
            while pending_c:
                cpr, cqc, cets = pending_c.pop(0)
                phase_ctx(cpr, cqc, cets)
            op_queue.extend(range(12, 16))
            for sc_ in op_queue:
                outproj_piece(sc_)

    nc.compile()
    return nc


def _split8(x, scale):
    import ml_dtypes
    f8 = ml_dtypes.float8_e4m3
    xs = (x * scale).astype(np.float32)
    hi = xs.astype(f8)
    lo = (xs - hi.astype(np.float32)).astype(f8)
    return hi, lo


def _prep_inputs(query, key, value, Wq, Wk, Wv, Wo):
    """Per-core input maps; host does transposes, scaling, fp8 splits."""
    import ml_dtypes
    f8 = ml_dtypes.float8_e4m3

    def pack_x(x):
        # [S, D] -> xT [D, S] -> scaled hi/lo fp8 [D, 2, S]
        xT = np.ascontiguousarray(x.T)
        hi, lo = _split8(xT, SX)
        o = np.empty((D, 2, S), f8)
        o[:, 0], o[:, 1] = hi, lo
        return o

    def pack_w(Wrows):
        # Wrows [DHG, D]; device wants W^T [D, DHG] scaled hi/lo [D, 2, DHG]
        wT = np.ascontiguousarray(Wrows.T)
        hi, lo = _split8(wT, SW)
        o = np.empty((D, 2, DHG), f8)
        o[:, 0], o[:, 1] = hi, lo
        return o

    in_maps = []
    per_hg = {}
    for hg in range(2):
        lo_, hi_ = hg * DHG, (hg + 1) * DHG
        per_hg[hg] = {
            "wq8": pack_w(Wq[lo_:hi_, :]),
            "wk8": pack_w(Wk[lo_:hi_, :]),
            "wv8": pack_w(Wv[lo_:hi_, :]),
            "woT": np.ascontiguousarray(Wo[:, lo_:hi_].T).astype(np.float16),
        }
    per_b = {}
    for b in range(B):
        per_b[b] = {
            "xq8": pack_x(query[b]),
            "xk8": pack_x(key[b]),
            "xv8": pack_x(value[b]),
        }
    for c in range(N_CORES):
        b, hg = c // 2, c % 2
        in_maps.append({**per_b[b], **per_hg[hg]})
    return in_maps


def _reference_numpy(query, key, value, mask, Wq, Wk, Wv, Wo):
    """Correctness fallback for inputs the fast path doesn't handle."""
    out = np.empty((B, S, D), np.float32)
    for b in range(B):
        q = (query[b] @ Wq.T).reshape(S, H, HD).transpose(1, 0, 2)
        k = (key[b] @ Wk.T).reshape(S, H, HD).transpose(1, 0, 2)
        v = (value[b] @ Wv.T).reshape(S, H, HD).transpose(1, 0, 2)
        scores = np.einsum("hqd,hkd->hqk", q, k) / np.sqrt(np.float32(HD))
        scores = np.where(mask[b][None, :, :] == 0, -np.inf, scores)
        scores = scores - scores.max(axis=-1, keepdims=True)
        e = np.exp(scores)
        attn = e / e.sum(axis=-1, keepdims=True)
        ctx = np.einsum("hqk,hkd->hqd", attn, v)
        out[b] = ctx.transpose(1, 0, 2).reshape(S, D) @ Wo.T
    return out


def run_device(query, key, value, Wq, Wk, Wv, Wo, trace=False,
               trace_kwargs=None, debug=False):
    from concourse.bass_utils import run_bass_kernel_spmd

    key_ = ("nc", debug)
    if key_ not in _CACHE:
        _CACHE[key_] = _build_nc(debug)
    nc = _CACHE[key_]
    in_maps = _prep_inputs(query, key, value, Wq, Wk, Wv, Wo)
    res = run_bass_kernel_spmd(nc, in_maps, list(range(N_CORES)),
                               trace=trace, **(trace_kwargs or {}))
    out = np.empty((B, S, D), np.float32)
    for b in range(B):
        out[b] = res.results[2 * b]["out"] + res.results[2 * b + 1]["out"]
    return out, res


def kernel(query, key, value, mask, Wq, Wk, Wv, Wo):
    query = np.asarray(query, np.float32)
    key = np.asarray(key, np.float32)
    value = np.asarray(value, np.float32)
    Wq = np.asarray(Wq, np.float32)
    Wk = np.asarray(Wk, np.float32)
    Wv = np.asarray(Wv, np.float32)
    Wo = np.asarray(Wo, np.float32)
    if not np.all(np.asarray(mask) == 1):
        return _reference_numpy(query, key, value, np.asarray(mask),
                                Wq, Wk, Wv, Wo)
    out, _ = run_device(query, key, value, Wq, Wk, Wv, Wo)
    return out


# revision 33
# speedup vs baseline: 1.3179x; 1.0500x over previous
"""Multi-head attention (B=4, S=2048, D=1024, H=16) on 8 Trainium2 cores.

Sharding: core c computes batch b = c // 2, head group hg = c % 2 (8 heads).
Each core runs the full pipeline for its (b, hg); the host sums the two
partial out-projection results per batch (linear in the head dim).

Kernel structure (per core):
  * QKV projections in fp8 (e4m3) DoubleRow matmuls: host pre-splits x (*4)
    and W (*128) into hi+lo fp8 pairs; each DoubleRow instruction carries two
    rank-128 contraction slots, and the hi*hi / lo*hi / hi*lo products (lo*lo
    dropped, ~2.6e-4 relative) pack 3 slots per 2 contraction chunks ->
    0.75x the fp16 PE cost. PSUM = 512*(x@W); evictions scale by 1/512.
  * Scores stay fp16 (single rank-64 chunk per output block: DoubleRow slot
    packing cannot help), computed as S^T [k, q] per head pair, exp on ACT
    -> E^T tiles [128k, 2h*512q] fp16 (with a constant ln(1/ALPHA) bias so
    the optional DVE exp path matches; the softmax denominator cancels it).
  * Context via stationary-E orientation: out ctx [128 q, 65] = E-chunk^T
    [k,128q]-stationary x [V | ones] moving -> PE cost 65 cycles per
    rank-128 key block (vs 512 in the V-stationary orientation). PSUM col 64
    accumulates the softmax denominator; normalization is a [128,1]
    reciprocal + per-partition scalar multiply, written as [q, 2h*64] f16
    and DMA-transposed (xbar) into the ctxT [d, q] layout the out-projection
    needs as its stationary operand.
  * Out-projection fp16.
  * Emission scheduling: PE is strictly in-order and the exp pipeline is
    paced by 2 score-PSUM buffers, so all non-score PE work (projections,
    ctx groups, out-projection) is split into ~0.1-0.45us micro-units and
    drained between score matmuls by a budget scheduler (~0.6us/key-block,
    deficit carryover).  Phases walk pr-major so projection deadlines
    spread evenly; a release-step gate paces projection DMA; ctx units gate
    on kb>=2 so they never wait on the previous phase's last exp.
  * An alternative exp path (int32 Schraudolph bit-trick + quadratic
    mantissa correction, 5 standard DVE ops) is implemented and HW-verified
    (~0.2% max err) but disabled (dve_kbs=()): the PE<->ACT coupling cost
    exceeded the ACT relief in TimelineSim.
"""

import numpy as np

B, S, D, H = 4, 2048, 1024, 16
HD = D // H          # 64
NHL = 8              # heads per core
DHG = NHL * HD       # 512 head-group width
HDA = HD + 1         # augmented head dim (ones column)
P = 128
N_CORES = 8

SX = 4.0             # host fp8 pre-scale for activations
SW = 128.0           # host fp8 pre-scale for weights
SEV = 1.0 / (SX * SW)  # eviction scale restoring natural units

# DVE-path exp (Schraudolph bit-trick with quadratic mantissa correction):
# z = int32(2^23*(log2e*s/8 + 127)); y = 2^i*(m^2 + BETA*m + GAMMA) where
# m = 1+frac in [1,2) via mantissa mask/or; y ~= (1/ALPHA)*e^(s/8) with
# 0.2% max error.  The ACT path applies bias ln(1/ALPHA) so both engines
# produce identically scaled E; the softmax denominator cancels the scale.
EXP_BETA = -0.049435831132835006
EXP_GAMMA = 2.020485350629873
EXP_ALPHA = 0.3371619879706471
EXP_A = float(2 ** 23 * np.log2(np.e) / 8.0)
EXP_B = float(2 ** 23 * 127)
EXP_BIAS = float(-np.log(EXP_ALPHA))

_CACHE = {}


def _build_nc(debug=False):
    import concourse.bacc as bacc
    import concourse.mybir as mybir
    from concourse.tile import TileContext

    f8 = mybir.dt.float8e4
    f16 = mybir.dt.float16
    f32 = mybir.dt.float32
    i32 = mybir.dt.int32
    EXP = mybir.ActivationFunctionType.Exp
    DR = mybir.MatmulPerfMode.DoubleRow
    ALU = mybir.AluOpType

    nc = bacc.Bacc("TRN2", target_bir_lowering=False, debug=False,
                   num_devices=N_CORES)

    # fp8 hi/lo packed inputs: [...,2,...] pair dim innermost-but-one.
    xq8 = nc.dram_tensor("xq8", [D, 2, S], f8, kind="ExternalInput")
    xk8 = nc.dram_tensor("xk8", [D, 2, S], f8, kind="ExternalInput")
    xv8 = nc.dram_tensor("xv8", [D, 2, S], f8, kind="ExternalInput")
    wq8 = nc.dram_tensor("wq8", [D, 2, DHG], f8, kind="ExternalInput")
    wk8 = nc.dram_tensor("wk8", [D, 2, DHG], f8, kind="ExternalInput")
    wv8 = nc.dram_tensor("wv8", [D, 2, DHG], f8, kind="ExternalInput")
    woT = nc.dram_tensor("woT", [DHG, D], f16, kind="ExternalInput")
    out = nc.dram_tensor("out", [S, D], f32, kind="ExternalOutput")

    DT = D // P          # 8 contraction chunks for projections
    PT = DHG // P        # 4 head-pair tiles
    QC = S // 512        # 4 query chunks
    KB = S // P          # 16 key blocks
    QS = 4               # 128-query sub-chunks per qc

    with TileContext(nc) as tc:
        with (
            tc.tile_pool(name="weights", bufs=1) as wpool,
            tc.tile_pool(name="persist", bufs=1) as persist,
            tc.tile_pool(name="xstream", bufs=5) as xpool,
            tc.tile_pool(name="etile", bufs=32) as etpool,
            tc.tile_pool(name="evict", bufs=6) as epool,
            tc.tile_pool(name="norm", bufs=10) as npool,
            tc.tile_pool(name="ctxsb", bufs=5) as cspool,
            tc.tile_pool(name="expsc", bufs=1) as xppool,
            tc.tile_pool(name="proj_psum", bufs=2, space="PSUM") as proj_psum,
            tc.tile_pool(name="sc_psum", bufs=2, space="PSUM") as sc_psum,
            tc.tile_pool(name="ctx_psum", bufs=2, space="PSUM") as ctx_psum,
        ):
            # -- weights (loads deferred/interleaved by the scheduler) ----
            wq = [wpool.tile([P, DT, DHG], f8, name=f"wq{i}") for i in range(2)]
            wk = [wpool.tile([P, DT, DHG], f8, name=f"wk{i}") for i in range(2)]
            wv = [wpool.tile([P, DT, DHG], f8, name=f"wv{i}") for i in range(2)]
            wo = wpool.tile([P, PT, D], f16)

            def load_w(wt, wd):
                for i in range(2):
                    nc.sync.dma_start(
                        wt[i][:],
                        wd[:, i, :].rearrange("(dt p) m -> p dt m", p=P))

            def load_wo():
                nc.sync.dma_start(
                    wo[:], woT[:].rearrange("(pt p) m -> p pt m", p=P))

            bias_t = persist.tile([P, 1], f32)
            nc.any.memset(bias_t[:], EXP_BIAS)
            kT = persist.tile([P, PT, S], f16)
            qT = persist.tile([P, PT, S], f16)
            vaug = persist.tile([P, KB, NHL * HDA], f16)
            ctxT = persist.tile([P, PT, S], f16)
            for kb in range(KB):
                nc.any.memset(
                    vaug[:, kb].rearrange("p (h x) -> p h x", h=NHL)[:, :, HD:HDA],
                    1.0)

            def load_x(xdram, qc):
                """x chunk hi/lo pair: two [128, DT, 512] fp8 tiles."""
                xts = []
                for i in range(2):
                    xt = xpool.tile([P, DT, 512], f8, tag="x", name=f"x{i}")
                    nc.sync.dma_start(
                        xt[:],
                        xdram[:, i, qc * 512:(qc + 1) * 512]
                        .rearrange("(dt p) n -> p dt n", p=P))
                    xts.append(xt)
                return xts

            # The 12 DoubleRow instructions of one projection tile, split
            # into micro-slices for the scheduler.  Slot i covers
            # instructions 4i..4i+4 of: for each chunk pair c, the
            # (hi*hi, lo*hi, hi*lo) slot products.
            _MM3 = [(c, wi, xi) for c in range(0, DT, 2)
                    for (wi, xi) in ((0, 0), (1, 0), (0, 1))]

            def mm3_part(ps, w8, x8, mslice, part):
                """Stationary = w8 slice, moving = x8 (projections Q/K)."""
                for i in range(4 * part, 4 * part + 4):
                    c, wi, xi = _MM3[i]
                    nc.tensor.matmul(
                        ps[:], w8[wi][:, c:c + 2, mslice],
                        x8[xi][:, c:c + 2, :],
                        start=(i == 0), stop=(i == len(_MM3) - 1),
                        perf_mode=DR)

            def mm3v_part(ps, x8, ks, hp, part):
                """Stationary = x8 key-block slice, moving = wv (V proj)."""
                for i in range(4 * part, 4 * part + 4):
                    c, xi, wi = _MM3[i]
                    nc.tensor.matmul(
                        ps[:, 0:P], x8[xi][:, c:c + 2, ks * P:(ks + 1) * P],
                        wv[wi][:, c:c + 2, hp * P:(hp + 1) * P],
                        start=(i == 0), stop=(i == len(_MM3) - 1),
                        perf_mode=DR)

            # -- attention phase pieces -----------------------------------
            _csb_stash = {}

            def ctx_group(pr, qc, ets, h, qs):
                """One ctx accumulation group [128q, 65] + normalization."""
                hg = 2 * pr + h
                cps = ctx_psum.tile([P, 512], f32, tag="ctx")
                for kb in range(KB):
                    nc.tensor.matmul(
                        cps[:, 0:HDA],
                        ets[kb][:, h * 512 + qs * P:h * 512 + (qs + 1) * P],
                        vaug[:, kb, hg * HDA:(hg + 1) * HDA],
                        start=(kb == 0), stop=(kb == KB - 1))
                rec = npool.tile([P, 1], f32, tag="rec")
                nc.vector.reciprocal_approx_fast(rec[:], cps[:, HD:HDA])
                if h == 0:
                    csb = cspool.tile([P, P], f16, tag="csb",
                                      name=f"csb_{pr}_{qc}_{qs}")
                    _csb_stash[(pr, qc, qs)] = csb
                else:
                    csb = _csb_stash.pop((pr, qc, qs))
                nc.vector.tensor_scalar_mul(
                    csb[:, h * HD:(h + 1) * HD], cps[:, 0:HD], rec[:])
                if h == 1:
                    nc.sync.dma_start_transpose(
                        ctxT[:, pr, qc * 512 + qs * P:qc * 512 + (qs + 1) * P],
                        csb[:])

            def push_op_micro(sc_, jc, min_kb):
                hold = {}
                def f1():
                    hold['ps'] = proj_psum.tile([P, 512], f32, tag="proj",
                                                name="ps_op")
                    for dt in (0, 1):
                        nc.tensor.matmul(
                            hold['ps'][:], ctxT[:, dt, sc_ * P:(sc_ + 1) * P],
                            wo[:, dt, jc * 512:(jc + 1) * 512],
                            start=(dt == 0), stop=False)
                def f2():
                    for dt in (2, 3):
                        nc.tensor.matmul(
                            hold['ps'][:], ctxT[:, dt, sc_ * P:(sc_ + 1) * P],
                            wo[:, dt, jc * 512:(jc + 1) * 512],
                            start=False, stop=(dt == PT - 1))
                    ot = epool.tile([P, 512], f32, tag="o")
                    nc.vector.tensor_copy(ot[:], hold['ps'][:])
                    nc.sync.dma_start(
                        out[sc_ * P:(sc_ + 1) * P, jc * 512:(jc + 1) * 512],
                        ot[:])
                work_q.append((427, f1, None, min_kb))
                work_q.append((427, f2, None, min_kb))

            def outproj_half(sc_, jc):
                ps = proj_psum.tile([P, 512], f32, tag="proj")
                for dt in range(PT):
                    nc.tensor.matmul(
                        ps[:], ctxT[:, dt, sc_ * P:(sc_ + 1) * P],
                        wo[:, dt, jc * 512:(jc + 1) * 512],
                        start=(dt == 0), stop=(dt == PT - 1))
                ot = epool.tile([P, 512], f32, tag="o")
                nc.vector.tensor_copy(ot[:], ps[:])
                nc.sync.dma_start(
                    out[sc_ * P:(sc_ + 1) * P, jc * 512:(jc + 1) * 512],
                    ot[:])

            # -- scheduler ------------------------------------------------
            # PE is strictly in-order and sc_psum is only 2 deep, so the
            # score loop runs at ACT's exp pace (~1.04us/kb) leaving
            # ~0.6us/kb of PE slack.  All other PE work (projections, ctx
            # groups, out-projection) is queued as small units and drained
            # between kb steps so neither engine ever waits on the other.
            # work_q (ctx/outproj, FIFO) drains before script_q
            # (projections, ordered with markers for dependency forcing).
            work_q = []      # entries: (ns, fn, marker, min_kb)
            script_q = []    # entries: (ns, fn, marker)
            done_markers = set()
            _bud = [0.0]

            cur_step = [99]

            def drain(add, kb=99):
                # Budget accumulator with deficit carryover: a 1280ns unit
                # drained against a 600ns slot leaves a debt the next slots
                # repay, keeping PE on ACT's pace on average.  work_q units
                # gated on min_kb (ctx groups touch the previous phase's
                # last exp output, which ACT only finishes ~1 kb into this
                # phase - draining them earlier stalls PE on ACT and
                # starves the exp pipeline).
                _bud[0] += add
                while _bud[0] > 0:
                    if work_q and kb >= work_q[0][3]:
                        ns, fn, mk = work_q.pop(0)[:3]
                    elif script_q and script_q[0][3] <= cur_step[0]:
                        ns, fn, mk = script_q.pop(0)[:3]
                    else:
                        break
                    fn()
                    if mk:
                        done_markers.add(mk)
                    _bud[0] -= ns
                if _bud[0] > 1200:
                    _bud[0] = 1200.0

            def force(mk):
                while mk not in done_markers:
                    ns, fn, m2, _rel = script_q.pop(0)
                    fn()
                    if m2:
                        done_markers.add(m2)

            def exp_dve_tail(zt, et):
                m2 = xppool.tile([P, 1024], i32, tag="xm2", name="m2")
                nc.vector.tensor_scalar(m2[:], zt[:], 0x007FFFFF, 0x3F800000,
                                        ALU.bitwise_and, ALU.bitwise_or)
                a1 = xppool.tile([P, 1024], i32, tag="xa1", name="a1")
                nc.vector.tensor_scalar(a1[:], zt[:], -8388608, 0,
                                        ALU.bitwise_and, ALU.bitwise_or)
                t = xppool.tile([P, 1024], f32, tag="xt", name="t")
                nc.vector.scalar_tensor_tensor(
                    t[:], m2[:].bitcast(f32), EXP_BETA, m2[:].bitcast(f32),
                    ALU.add, ALU.mult)
                nc.vector.scalar_tensor_tensor(
                    et[:], t[:], EXP_GAMMA, a1[:].bitcast(f32),
                    ALU.add, ALU.mult)

            def phase_scores(pr, qc, dve_kbs=()):
                ets = []
                for kb in range(KB):
                    sc = sc_psum.tile([P, 1024], f32, tag="sc")
                    et = etpool.tile([P, 1024], f16, tag="e",
                                     name=f"et_{pr}_{qc}_{kb}")
                    for h in range(2):
                        nc.tensor.matmul(
                            sc[:, h * 512:(h + 1) * 512],
                            kT[h * 64:(h + 1) * 64, pr, kb * P:(kb + 1) * P],
                            qT[h * 64:(h + 1) * 64, pr, qc * 512:(qc + 1) * 512],
                            start=True, stop=True)
                    if kb in dve_kbs:
                        zt = xppool.tile([P, 1024], i32, tag="xz", name="zt")
                        nc.vector.tensor_scalar(zt[:], sc[:], EXP_A, EXP_B,
                                                ALU.mult, ALU.add)
                        work_q.append(
                            (0, (lambda z, e: lambda: exp_dve_tail(z, e))
                             (zt, et), None, kb + 2))
                    else:
                        nc.scalar.activation(et[:], sc[:], EXP,
                                             scale=1.0 / 8.0, bias=bias_t[:])
                    ets.append(et)
                    drain(600, kb=kb)
                return ets

            # -- projection script ----------------------------------------
            # Micro-units (<=~430ns of PE each) keep filler jitter small
            # relative to the ~610ns/kb slack ACT leaves, so exp never
            # waits long for its next scores tile.  Loads sit ahead of
            # their consumers; x chunks for K rows pr>0 and the per-head-
            # pair V passes are reloaded (extra DMA, big SBUF saving).
            xslot = {}
            sq = script_q
            _rel = [0]

            def sq_load(key, xdram, qc):
                def f():
                    xslot[key] = load_x(xdram, qc)
                sq.append((0, f, None, _rel[0]))

            def sq_misc(f):
                sq.append((0, f, None, _rel[0]))

            def sq_kq(w8, key, dst, pt, qc, mk=None):
                hold = {}
                for part in range(3):
                    def f(part=part):
                        if part == 0:
                            hold['ps'] = proj_psum.tile(
                                [P, 512], f32, tag="proj", name="ps_u")
                        mm3_part(hold['ps'], w8, xslot[key],
                                 slice(pt * P, (pt + 1) * P), part)
                        if part == 2:
                            nc.vector.tensor_scalar_mul(
                                dst[:, pt, qc * 512:(qc + 1) * 512],
                                hold['ps'][:], SEV)
                    sq.append((427, f, mk if part == 2 else None, _rel[0]))

            def sq_v(key, kc, ks, hp, mk=None):
                hold = {}
                for part in range(3):
                    def f(part=part):
                        if part == 0:
                            hold['ps'] = proj_psum.tile(
                                [P, 512], f32, tag="proj", name="ps_u")
                        mm3v_part(hold['ps'], xslot[key], ks, hp, part)
                        if part == 2:
                            kb = kc * 4 + ks
                            va = vaug[:, kb].rearrange("p (h x) -> p h x",
                                                       h=NHL)
                            nc.vector.tensor_scalar_mul(
                                va[:, 2 * hp:2 * hp + 2, 0:HD],
                                hold['ps'][:, 0:P]
                                .rearrange("p (h x) -> p h x", h=2), SEV)
                    sq.append((107, f, mk if part == 2 else None, _rel[0]))

            # startup: only what the first score phase needs, DMA-ordered
            # so Q's operands land first.
            load_w(wq, wq8)
            xslot["q0"] = load_x(xq8, 0)
            load_w(wk, wk8)
            xslot["k0"] = load_x(xk8, 0)
            ps0 = proj_psum.tile([P, 512], f32, tag="proj", name="ps_q00")
            for part in range(3):
                mm3_part(ps0, wq, xslot["q0"], slice(0, P), part)
            nc.vector.tensor_scalar_mul(qT[:, 0, 0:512], ps0[:], SEV)
            ps1 = proj_psum.tile([P, 512], f32, tag="proj", name="ps_k00")
            for part in range(3):
                mm3_part(ps1, wk, xslot["k0"], slice(0, P), part)
            nc.vector.tensor_scalar_mul(kT[:, 0, 0:512], ps1[:], SEV)

            # script, in drain order aligned with the pr-major phase walk.
            # Deadlines: Q(pr,qc) at phase (pr,qc); K{pr} and Q(pr,0) at
            # row start (pr,0); V{pr} when C(pr,0) drains one phase later.
            # Row pr's K/V x chunks are reloaded per pass (DMA for SBUF).
            def k_pass(pt, mk, skip0=False):
                """K row pt with one-chunk load lookahead."""
                kcs = list(range(1 if skip0 else 0, QC))
                keys = [f"k{pt}_{kc}" for kc in kcs]
                sq_load(keys[0], xk8, kcs[0])
                for j, kc in enumerate(kcs):
                    if j + 1 < len(kcs):
                        sq_load(keys[j + 1], xk8, kcs[j + 1])
                    sq_kq(wk, keys[j], kT, pt, kc,
                          mk=mk if j == len(kcs) - 1 else None)

            def v_pass(hp, mk):
                sq_load(f"v{hp}_0", xv8, 0)
                for kc in range(QC):
                    if kc + 1 < QC:
                        sq_load(f"v{hp}_{kc + 1}", xv8, kc + 1)
                    for ks in range(4):
                        sq_v(f"v{hp}_{kc}", kc, ks, hp,
                             mk=mk if (kc, ks) == (QC - 1, 3) else None)

            _rel[0] = 0
            sq_misc(lambda: load_w(wv, wv8))
            k_pass(0, "K0", skip0=True)       # chunks 1-3 of row 0
            v_pass(0, "V0")
            sq_misc(load_wo)
            sq_load("qa1", xq8, 1)
            sq_kq(wq, "qa1", qT, 0, 1)
            sq_kq(wq, "qa1", qT, 1, 1, mk="QA1")
            _rel[0] = 1
            sq_load("qa2", xq8, 2)
            sq_kq(wq, "qa2", qT, 0, 2)
            sq_kq(wq, "qa2", qT, 1, 2, mk="QA2")
            _rel[0] = 2
            sq_load("qa3", xq8, 3)
            sq_kq(wq, "qa3", qT, 0, 3)
            sq_kq(wq, "qa3", qT, 1, 3, mk="QA3")
            sq_load("qb0", xq8, 0)
            sq_kq(wq, "qb0", qT, 1, 0)
            k_pass(1, "K1")
            _rel[0] = 4
            v_pass(1, "V1")
            _rel[0] = 5
            sq_load("qc0", xq8, 0)
            sq_kq(wq, "qc0", qT, 2, 0)
            sq_kq(wq, "qc0", qT, 3, 0)
            k_pass(2, "K2")
            _rel[0] = 7
            v_pass(2, "V2")
            _rel[0] = 8
            sq_load("qd1", xq8, 1)
            sq_kq(wq, "qd1", qT, 2, 1)
            sq_kq(wq, "qd1", qT, 3, 1, mk="QB1")
            sq_load("qd2", xq8, 2)
            sq_kq(wq, "qd2", qT, 2, 2)
            sq_kq(wq, "qd2", qT, 3, 2, mk="QB2")
            _rel[0] = 9
            sq_load("qd3", xq8, 3)
            sq_kq(wq, "qd3", qT, 2, 3)
            sq_kq(wq, "qd3", qT, 3, 3, mk="QB3")
            _rel[0] = 10
            k_pass(3, "K3")
            _rel[0] = 11
            v_pass(3, "V3")

            # -- main loop (pr-major) -------------------------------------
            def qmarker(pr, qc):
                if qc == 0:
                    return None        # covered by startup / K-row forcing
                return f"QA{qc}" if pr <= 1 else f"QB{qc}"

            prev = None
            for pr in range(PT):
                for qc in range(QC):
                    cur_step[0] = 4 * pr + qc
                    if pr > 0 and qc == 0:
                        force(f"K{pr}")
                    mk = qmarker(pr, qc)
                    if mk:
                        force(mk)
                    if prev is not None:
                        ppr, pqc, pets = prev
                        if pqc == 0:
                            force(f"V{ppr}")
                        for h in range(2):
                            for qs in range(QS):
                                work_q.append(
                                    (450, (lambda a, b, c, d, e:
                                           lambda: ctx_group(a, b, c, d, e))
                                     (ppr, pqc, pets, h, qs), None, 2))
                        if ppr == 3 and pqc < QC - 1:
                            for sc_ in range(pqc * 4, pqc * 4 + 4):
                                for jc in range(2):
                                    push_op_micro(sc_, jc, 2)
                    dkb = ()
                    ets = phase_scores(pr, qc, dve_kbs=dkb)
                    prev = (pr, qc, ets)
            # tail: last ctx phase + remaining outproj
            ppr, pqc, pets = prev
            for h in range(2):
                for qs in range(QS):
                    work_q.append(
                        (450, (lambda a, b, c, d, e:
                               lambda: ctx_group(a, b, c, d, e))
                         (ppr, pqc, pets, h, qs), None, 0))
            for sc_ in range(12, 16):
                for jc in range(2):
                    push_op_micro(sc_, jc, 0)
            while work_q or script_q:
                drain(10**9)

    nc.compile()
    return nc


def _split8(x, scale):
    import ml_dtypes
    f8 = ml_dtypes.float8_e4m3
    xs = (x * scale).astype(np.float32)
    hi = xs.astype(f8)
    lo = (xs - hi.astype(np.float32)).astype(f8)
    return hi, lo


def _prep_inputs(query, key, value, Wq, Wk, Wv, Wo):
    """Per-core input maps; host does transposes, scaling, fp8 splits."""
    import ml_dtypes
    f8 = ml_dtypes.float8_e4m3

    def pack_x(x):
        # [S, D] -> xT [D, S] -> scaled hi/lo fp8 [D, 2, S]
        xT = np.ascontiguousarray(x.T)
        hi, lo = _split8(xT, SX)
        o = np.empty((D, 2, S), f8)
        o[:, 0], o[:, 1] = hi, lo
        return o

    def pack_w(Wrows):
        # Wrows [DHG, D]; device wants W^T [D, DHG] scaled hi/lo [D, 2, DHG]
        wT = np.ascontiguousarray(Wrows.T)
        hi, lo = _split8(wT, SW)
        o = np.empty((D, 2, DHG), f8)
        o[:, 0], o[:, 1] = hi, lo
        return o

    in_maps = []
    per_hg = {}
    for hg in range(2):
        lo_, hi_ = hg * DHG, (hg + 1) * DHG
        per_hg[hg] = {
            "wq8": pack_w(Wq[lo_:hi_, :]),
            "wk8": pack_w(Wk[lo_:hi_, :]),
            "wv8": pack_w(Wv[lo_:hi_, :]),
            "woT": np.ascontiguousarray(Wo[:, lo_:hi_].T).astype(np.float16),
        }
    per_b = {}
    for b in range(B):
        per_b[b] = {
            "xq8": pack_x(query[b]),
            "xk8": pack_x(key[b]),
            "xv8": pack_x(value[b]),
        }
    for c in range(N_CORES):
        b, hg = c // 2, c % 2
        in_maps.append({**per_b[b], **per_hg[hg]})
    return in_maps


def _reference_numpy(query, key, value, mask, Wq, Wk, Wv, Wo):
    """Correctness fallback for inputs the fast path doesn't handle."""
    out = np.empty((B, S, D), np.float32)
    for b in range(B):
        q = (query[b] @ Wq.T).reshape(S, H, HD).transpose(1, 0, 2)
        k = (key[b] @ Wk.T).reshape(S, H, HD).transpose(1, 0, 2)
        v = (value[b] @ Wv.T).reshape(S, H, HD).transpose(1, 0, 2)
        scores = np.einsum("hqd,hkd->hqk", q, k) / np.sqrt(np.float32(HD))
        scores = np.where(mask[b][None, :, :] == 0, -np.inf, scores)
        scores = scores - scores.max(axis=-1, keepdims=True)
        e = np.exp(scores)
        attn = e / e.sum(axis=-1, keepdims=True)
        ctx = np.einsum("hqk,hkd->hqd", attn, v)
        out[b] = ctx.transpose(1, 0, 2).reshape(S, D) @ Wo.T
    return out


def run_device(query, key, value, Wq, Wk, Wv, Wo, trace=False,
               trace_kwargs=None, debug=False):
    from concourse.bass_utils import run_bass_kernel_spmd

    key_ = ("nc", debug)
    if key_ not in _CACHE:
        _CACHE[key_] = _build_nc(debug)
    nc = _CACHE[key_]
    in_maps = _prep_inputs(query, key, value, Wq, Wk, Wv, Wo)
    res = run_bass_kernel_spmd(nc, in_maps, list(range(N_CORES)),
                               trace=trace, **(trace_kwargs or {}))
    out = np.empty((B, S, D), np.float32)
    for b in range(B):
        out[b] = res.results[2 * b]["out"] + res.results[2 * b + 1]["out"]
    return out, res


def kernel(query, key, value, mask, Wq, Wk, Wv, Wo):
    query = np.asarray(query, np.float32)
    key = np.asarray(key, np.float32)
    value = np.asarray(value, np.float32)
    Wq = np.asarray(Wq, np.float32)
    Wk = np.asarray(Wk, np.float32)
    Wv = np.asarray(Wv, np.float32)
    Wo = np.asarray(Wo, np.float32)
    if not np.all(np.asarray(mask) == 1):
        return _reference_numpy(query, key, value, np.asarray(mask),
                                Wq, Wk, Wv, Wo)
    out, _ = run_device(query, key, value, Wq, Wk, Wv, Wo)
    return out
